# revision 1
# baseline (speedup 1.0000x reference)
"""Bass/Tile kernel for nn_EncoderBlock (dense transformer w/ graph-masked
attention + GIN MLP). Per-core program: 2 batches, L=512, C=512, H=4, HS=128,
HID=2048. Data-parallel over batch across 8 cores, no collectives.

Layout strategy (per batch):
  - LN stats in token-major (bn_stats), center/scale via tensor_scalar,
    PE-transpose to channel-major, fuse ln gamma/beta into the transpose
    copyback (per-partition scalars there).
  - qT,kT channel-major [C,L]; v token-major [L,C]  (straight matmuls from
    xn1T, no extra transposes; per-head slices are single tiles).
  - scores computed TRANSPOSED: scoreT[lk,lq] = kT_chunk.T @ qT. Mask applied
    as a -57344 bias accumulated into score PSUM via (-57344*I) @ comp fp8e5
    matmul. exp via ACT straight from PSUM (scale=1/sqrt(HS) folded in).
    Softmax denominator via ones-lhsT matmuls; normalization fused into the
    attention-output PSUM->SBUF copyback with a partition_broadcast recip.
    Heads processed in pairs so exp/denoms/attn-out pipeline across heads.
  - attn-out matmuls need no transposes: lhsT = v token-major chunks.
  - proj produces y token-major directly (lhsT = OT chunks), residual fused
    into copyback.
  - GIN: g token-major (lhsT=xn2T), hT=fc1+z computed hid-major with z
    matmuls (lhsT=g chunks, rhs=adjT/adj) accumulated into the same PSUM
    bank as fc1, relu on copyback. fc2 from hT (lhsT) + residual on copyback.
    adj prep + LN2 live in a separate pool emitted right after each batch's
    attention so they overlap the other batch's attention phase.
  - masks: a = (|rel_pos-5|==4); m2=aTa, m3=aaT via fp8e4 DoubleRow matmuls
    (binary values exact, K=256/mm). Mask complements binarized via is_lt
    with the +I diagonal handled by zeroing the complement diagonal via
    affine_select. Transposed masks free: compT(h0)=comp1, compT(h1)=comp0,
    h2,h3 symmetric.
"""

import sys
for _p in ("/opt/trn_rl_repo", "/root/.axon_site/_ro/trn_rl_repo"):
    if _p not in sys.path:
        sys.path.append(_p)

from contextlib import ExitStack

import concourse.bass as bass
import concourse.tile as tile
from concourse import mybir
from concourse.bass import ts
from concourse.masks import make_identity

F32 = mybir.dt.float32
F32R = mybir.dt.float32r
BF16 = mybir.dt.bfloat16
FP8 = mybir.dt.float8e4
FP8E5 = mybir.dt.float8e5
I32 = mybir.dt.int32
OP = mybir.AluOpType
ACT = mybir.ActivationFunctionType

P = 128
L = 512
C = 512
H = 4
HS = 128
HID = 2048
NB = 2          # batches per core
LC = L // P     # 4 token chunks
CC = C // P     # 4 channel chunks
HC = HID // P   # 16 hidden chunks
EPS = 1e-5
INV_SQRT_HS = 1.0 / (HS ** 0.5)
NEG8 = -57344.0   # most negative finite fp8e5; * scale it still floors exp to 0


def build_encoder_program(nc):
    """Emit the full 2-batch encoder program into `nc`."""
    def dram(name, shape, kind):
        return nc.dram_tensor(name, shape, F32, kind=kind).ap()

    x_d = dram("x", [NB, L, C], "ExternalInput")
    rp_d = dram("rel_pos", [NB, L, L], "ExternalInput")
    adj_d = dram("adj", [NB, L, L], "ExternalInput")
    wqkv_d = dram("w_qkv", [C, 3 * C], "ExternalInput")
    wproj_d = dram("w_proj", [C, C], "ExternalInput")
    ln1g_d = dram("ln1_g", [C], "ExternalInput")
    ln1b_d = dram("ln1_b", [C], "ExternalInput")
    ln2g_d = dram("ln2_g", [C], "ExternalInput")
    ln2b_d = dram("ln2_b", [C], "ExternalInput")
    wfc1_d = dram("w_fc1", [C, HID], "ExternalInput")
    wgcn_d = dram("w_gcn", [C, HID], "ExternalInput")
    wfc2_d = dram("w_fc2", [HID, C], "ExternalInput")
    out_d = dram("out", [NB, L, C], "ExternalOutput")

    x_t3 = [x_d[b].rearrange("(lo p) c -> p lo c", p=P) for b in range(NB)]
    rp_t3 = [rp_d[b].rearrange("(lo p) c -> p lo c", p=P) for b in range(NB)]
    adj_t3 = [adj_d[b].rearrange("(lo p) c -> p lo c", p=P) for b in range(NB)]
    out_t3 = [out_d[b].rearrange("(lo p) c -> p lo c", p=P) for b in range(NB)]

    with ExitStack() as top:
        tc = top.enter_context(tile.TileContext(nc))
        const = top.enter_context(tc.tile_pool(name="const", bufs=1))
        persist = top.enter_context(tc.tile_pool(name="persist", bufs=1))
        ginpre = top.enter_context(tc.tile_pool(name="ginpre", bufs=1))
        psum = top.enter_context(tc.tile_pool(name="psum", bufs=1, space="PSUM"))
        attn_stack = ExitStack()
        wA = attn_stack.enter_context(tc.tile_pool(name="wA", bufs=1))
        ap = attn_stack.enter_context(tc.tile_pool(name="attn", bufs=1))

        def pmm():
            return psum.tile([P, 512], F32, tag="mm", bufs=4, name="pmm")

        def ptp(dt):
            return psum.tile([P, P], dt, tag="tp", bufs=3, name="ptp")

        # ---- input DMAs first: head of the DMA queues ----
        wq = wA.tile([P, CC, 3 * C], F32R)
        wp = wA.tile([P, CC, C], F32R)
        x_t0 = ap.tile([P, LC, C], F32, tag="x_t", bufs=2, name="x_t")
        for i in range(LC):
            nc.sync.dma_start(out=x_t0[:, i, :], in_=x_t3[0][:, i, :])
        ln_rows = {}
        for nm, dv in (("ln1g", ln1g_d), ("ln1b", ln1b_d),
                       ("ln2g", ln2g_d), ("ln2b", ln2b_d)):
            row = ap.tile([1, C], F32, tag="lnrow", bufs=2, name=f"{nm}_row")
            nc.sync.dma_start(out=row[:], in_=dv[None, :])
            ln_rows[nm] = row
        rel0 = []
        for i in range(LC):
            r = ap.tile([P, L], F32, tag="rel", bufs=2, name="rel")
            nc.sync.dma_start(out=r[:], in_=rp_t3[0][:, i, :])
            rel0.append(r)
        nc.sync.dma_start(
            out=wq[:],
            in_=wqkv_d.rearrange("(ko p) n -> p ko n", p=P).bitcast(F32R))
        nc.sync.dma_start(
            out=wp[:],
            in_=wproj_d.rearrange("(ko p) n -> p ko n", p=P).bitcast(F32R))

        # ---------------- constants ----------------
        ident_f = const.tile([P, P], F32)
        make_identity(nc, ident_f[:])
        ident_r = const.tile([P, P], F32R)
        nc.vector.tensor_copy(out=ident_r[:], in_=ident_f[:])
        ident_b = const.tile([P, P], BF16)
        nc.vector.tensor_copy(out=ident_b[:], in_=ident_f[:])
        negI_8 = const.tile([P, P], FP8E5)
        nc.gpsimd.memset(negI_8[:], 0.0)
        nc.gpsimd.affine_select(out=negI_8[:], in_=negI_8[:],
                                compare_op=OP.not_equal, fill=NEG8,
                                base=0, pattern=[[-1, P]], channel_multiplier=1)
        ones_f = const.tile([P, 1], F32)
        nc.vector.memset(ones_f[:], 1.0)
        ones_r = const.tile([P, 1], F32R)
        nc.vector.tensor_copy(out=ones_r[:], in_=ones_f[:])
        eps_t = const.tile([P, 1], F32)
        nc.vector.memset(eps_t[:], EPS)
        neg5_t = const.tile([P, 1], F32)
        nc.vector.memset(neg5_t[:], -5.0)
        magic4 = const.tile([P, LC], I32)
        nc.vector.memset(magic4[:], 0x5F3759DF)

        # HAM warmup: dummy matmuls so the PE clock-gate opens during
        # the initial input DMAs (otherwise first real matmuls run at 1.2GHz)
        warm_rhs = const.tile([P, 512], F32R)
        nc.vector.tensor_copy(out=warm_rhs[:, 0:P], in_=ident_f[:])
        for _ in range(14):
            pw = pmm()
            nc.tensor.matmul(pw[:], ident_r[:], warm_rhs[:], start=True, stop=True)

        # x1 residual stream (kept across phases)
        x1 = [persist.tile([P, LC, C], F32, name=f"x1_{b}", tag=f"x1_{b}")
              for b in range(NB)]

        # ---------------- layernorm helper ----------------
        def layer_norm_T(pool, xin, g_sb, b_sb, tag, out_dt=F32R,
                         keep_xc=False):
            """xin: [P, LC, C] token-major F32. Returns (xnT, xc_b): xnT
            [P, CC, L] with gamma/beta applied (fused into the transpose
            copyback); xc_b [P, LC, C] bf16 normalized token-major WITHOUT
            gamma/beta (only when keep_xc)."""
            xnT = pool.tile([P, CC, L], out_dt, tag=f"xnT_{tag}", name="xnT")
            xc_full = None
            if keep_xc:
                xc_full = pool.tile([P, LC, C], BF16, tag=f"xc_{tag}",
                                    bufs=2, name="xc_full")
            mu4 = pool.tile([P, LC], F32, tag="ln_mu4", bufs=2, name="mu4")
            s4 = pool.tile([P, LC], F32, tag="ln_s4", bufs=2, name="s4")
            for i in range(LC):
                st6 = pool.tile([P, 6], F32, tag="ln_st6", bufs=2, name="st6")
                nc.vector.bn_stats(out=st6[:], in_=xin[:, i, :])
                mv = pool.tile([P, 2], F32, tag="ln_mv", bufs=2, name="mv")
                nc.vector.bn_aggr(out=mv[:], in_=st6[:])
                nc.vector.tensor_copy(out=mu4[:, i:i + 1], in_=mv[:, 0:1])
                nc.vector.tensor_scalar(out=s4[:, i:i + 1], in0=mv[:, 1:2],
                                        scalar1=EPS, scalar2=None, op0=OP.add)
            # istd = rsqrt(var+eps) via Quake seed + 3 Newton steps, all DVE
            y4 = pool.tile([P, LC], F32, tag="ln_y4", bufs=2, name="y4")
            t4 = pool.tile([P, LC], F32, tag="ln_t4", bufs=2, name="t4")
            nc.vector.tensor_scalar(out=t4[:].bitcast(I32), in0=s4[:].bitcast(I32),
                                    scalar1=1, scalar2=None,
                                    op0=OP.arith_shift_right)
            nc.vector.tensor_tensor(out=y4[:].bitcast(I32), in0=magic4[:],
                                    in1=t4[:].bitcast(I32), op=OP.subtract)
            for _ in range(2):
                nc.vector.tensor_tensor(out=t4[:], in0=y4[:], in1=y4[:], op=OP.mult)
                nc.vector.tensor_tensor(out=t4[:], in0=t4[:], in1=s4[:], op=OP.mult)
                nc.vector.tensor_scalar(out=t4[:], in0=t4[:], scalar1=-0.5,
                                        scalar2=1.5, op0=OP.mult, op1=OP.add)
                nc.vector.tensor_tensor(out=y4[:], in0=y4[:], in1=t4[:], op=OP.mult)
            for i in range(LC):
                if keep_xc:
                    nc.vector.tensor_scalar(out=xc_full[:, i, :],
                                            in0=xin[:, i, :],
                                            scalar1=mu4[:, i:i + 1],
                                            scalar2=y4[:, i:i + 1],
                                            op0=OP.subtract, op1=OP.mult)
                    for j in range(CC):
                        pt = ptp(BF16)
                        nc.tensor.transpose(pt[:], xc_full[:, i, ts(j, P)],
                                            ident_b[:])
                        nc.vector.tensor_scalar(out=xnT[:, j, ts(i, P)],
                                                in0=pt[:],
                                                scalar1=g_sb[:, j:j + 1],
                                                scalar2=b_sb[:, j:j + 1],
                                                op0=OP.mult, op1=OP.add)
                    continue
                xc = pool.tile([P, C], F32R, tag="ln_xc", bufs=2, name="xc")
                nc.vector.tensor_scalar(out=xc[:], in0=xin[:, i, :],
                                        scalar1=mu4[:, i:i + 1],
                                        scalar2=y4[:, i:i + 1],
                                        op0=OP.subtract, op1=OP.mult)
                for j in range(CC):      # channel chunk (partition of output)
                    pt = ptp(F32R)
                    nc.tensor.transpose(pt[:], xc[:, ts(j, P)], ident_r[:])
                    nc.vector.tensor_scalar(out=xnT[:, j, ts(i, P)],
                                            in0=pt[:].bitcast(F32),
                                            scalar1=g_sb[:, j:j + 1],
                                            scalar2=b_sb[:, j:j + 1],
                                            op0=OP.mult, op1=OP.add)
            return xnT, xc_full

        def zero_diag(ap_2d, m):
            """Zero the diagonal-block entries of comp chunk m in place."""
            nc.gpsimd.affine_select(out=ap_2d, in_=ap_2d,
                                    compare_op=OP.not_equal, fill=0.0,
                                    base=P * m, pattern=[[-1, L]],
                                    channel_multiplier=1)

        # ---------- GIN prerequisites (overlap other batch's attention) ----
        def gin_pre(b):
            adj_b = ginpre.tile([P, LC, L], BF16, tag="adj_b", bufs=2,
                                name="adj_b")
            for i in range(LC):
                stg = ginpre.tile([P, L], F32, tag="stage", bufs=2, name="stg")
                nc.sync.dma_start(out=stg[:], in_=adj_t3[b][:, i, :])
                nc.vector.tensor_copy(out=adj_b[:, i, :], in_=stg[:])
            adjT_b = ginpre.tile([P, LC, L], BF16, tag="adjT_b", bufs=2,
                                 name="adjT_b")
            for i in range(LC):
                for j in range(LC):
                    pt = ptp(BF16)
                    nc.tensor.transpose(pt[:], adj_b[:, i, ts(j, P)], ident_b[:])
                    nc.vector.tensor_copy(out=adjT_b[:, j, ts(i, P)], in_=pt[:])
            xn2T, xc2_b = layer_norm_T(ginpre, x1[b], ln2g, ln2b, "2",
                                       keep_xc=True)
            return adj_b, adjT_b, xn2T, xc2_b

        # ================= attention =================
        # ln params were DMAed as [1,512] rows (1 descriptor vs 512);
        # PE-transpose 128-slices into partition-major [128, CC]
        def load_ln_param(name):
            row = ln_rows[name]
            pg = psum.tile([P, CC], F32, tag="dn", bufs=1, name="pg")
            for j in range(CC):
                nc.tensor.transpose(pg[:, j:j + 1], row[:, ts(j, P)], ident_f[0:1, 0:1])
            out = const.tile([P, CC], F32, name=name)
            nc.vector.tensor_copy(out=out[:], in_=pg[:])
            return out

        ln1g = load_ln_param("ln1g")
        ln1b = load_ln_param("ln1b")
        ln2g = load_ln_param("ln2g")
        ln2b = load_ln_param("ln2b")

        gin_inputs = {}

        def attn_ln(b):
            # ---- x + LN1: earliest PE work of the batch ----
            if b == 0:
                x_t = x_t0
            else:
                x_t = ap.tile([P, LC, C], F32, tag="x_t", bufs=2, name="x_t")
                for i in range(LC):
                    nc.sync.dma_start(out=x_t[:, i, :], in_=x_t3[b][:, i, :])
            xn1T, _ = layer_norm_T(ap, x_t, ln1g, ln1b, "1")
            return x_t, xn1T

        def attn_phase(b, x_t, xn1T):

            # ---- hop mask: a = (|rel-5| == 4) ----
            a_8 = ap.tile([P, LC, L], FP8, tag="a_8", name="a_8")
            a_b = ap.tile([P, LC, L], BF16, tag="a_b", name="a_b")
            comp0 = ap.tile([P, LC, L], FP8E5, tag="comp0", name="comp0")
            for i in range(LC):
                if b == 0:
                    rel = rel0[i]
                else:
                    rel = ap.tile([P, L], F32, tag="rel", bufs=2, name="rel")
                    nc.sync.dma_start(out=rel[:], in_=rp_t3[b][:, i, :])
                tabs = ap.tile([P, L], F32, tag="tabs", bufs=1, name="tabs")
                nc.scalar.activation(out=tabs[:], in_=rel[:],
                                     func=ACT.Abs, bias=neg5_t[:], scale=1.0)
                nc.vector.tensor_scalar(out=a_b[:, i, :], in0=tabs[:],
                                        scalar1=4.0, scalar2=None,
                                        op0=OP.is_equal)
                nc.vector.tensor_scalar(out=a_8[:, i, :], in0=tabs[:],
                                        scalar1=4.0, scalar2=None,
                                        op0=OP.is_equal)
                nc.vector.tensor_scalar(out=comp0[:, i, :], in0=tabs[:],
                                        scalar1=4.0, scalar2=None,
                                        op0=OP.not_equal)
                zero_diag(comp0[:, i, :], i)
            # aT (bf16 transpose) + fp8 copy + comp1
            aT_8 = ap.tile([P, LC, L], FP8, tag="aT_8", name="aT_8")
            comp1 = ap.tile([P, LC, L], FP8E5, tag="comp1", name="comp1")
            for i in range(LC):
                for j in range(LC):
                    pt = ptp(BF16)
                    nc.tensor.transpose(pt[:], a_b[:, i, ts(j, P)], ident_b[:])
                    nc.vector.tensor_copy(out=aT_8[:, j, ts(i, P)], in_=pt[:])
            for i in range(LC):
                nc.vector.tensor_scalar(out=comp1[:, i, :], in0=aT_8[:, i, :],
                                        scalar1=0.5, scalar2=None,
                                        op0=OP.is_lt)
                zero_diag(comp1[:, i, :], i)

            # ---- qT, kT (channel-major), v (token-major) ----
            qT = ap.tile([P, CC, L], F32R, tag="qT", name="qT")
            kT = ap.tile([P, CC, L], F32R, tag="kT", name="kT")
            for dst, off in ((qT, 0), (kT, C)):
                for m in range(CC):
                    pm = pmm()
                    for k in range(CC):
                        nc.tensor.matmul(pm[:], wq[:, k, off + m * P:off + (m + 1) * P],
                                         xn1T[:, k, :],
                                         start=(k == 0), stop=(k == CC - 1))
                    nc.vector.tensor_copy(out=dst[:, m, :], in_=pm[:])
            v_sb = ap.tile([P, LC, C], F32R, tag="v_sb", name="v_sb")
            for m in range(LC):
                pm = pmm()
                for k in range(CC):
                    nc.tensor.matmul(pm[:], xn1T[:, k, ts(m, P)],
                                     wq[:, k, 2 * C:3 * C],
                                     start=(k == 0), stop=(k == CC - 1))
                nc.vector.tensor_copy(out=v_sb[:, m, :], in_=pm[:])

            # ---- m2 = aTa, m3 = aaT (fp8 DoubleRow) -> complements ----
            comp2 = ap.tile([P, LC, L], FP8E5, tag="comp2", name="comp2")
            comp3 = ap.tile([P, LC, L], FP8E5, tag="comp3", name="comp3")
            for (cm, src) in ((comp2, a_8), (comp3, aT_8)):
                for m in range(LC):
                    pm = pmm()
                    for k in range(LC // 2):
                        nc.tensor.matmul(pm[:],
                                         src[:, 2 * k:2 * k + 2, ts(m, P)],
                                         src[:, 2 * k:2 * k + 2, :],
                                         start=(k == 0), stop=(k == 1),
                                         perf_mode=mybir.MatmulPerfMode.DoubleRow)
                    nc.vector.tensor_scalar(out=cm[:, m, :], in0=pm[:],
                                            scalar1=0.5, scalar2=None,
                                            op0=OP.is_lt)
                    zero_diag(cm[:, m, :], m)

            # ---- attention heads (pairs pipeline) ----
            compT = [comp1, comp0, comp2, comp3]
            OT = ap.tile([P, H, L], F32R, tag="OT", name="OT")
            for pair in ((0, 1), (2, 3)):
                atts = {}
                rbcs = {}
                for h in pair:
                    attnT = ap.tile([P, LC, L], F32R, tag="attnT", bufs=3,
                                    name="attnT")
                    atts[h] = attnT
                    for i in range(LC):
                        pm = pmm()
                        nc.tensor.matmul(pm[:], kT[:, h, ts(i, P)], qT[:, h, :],
                                         start=True, stop=False)
                        nc.tensor.matmul(pm[:], negI_8[:], compT[h][:, i, :],
                                         start=False, stop=True)
                        nc.scalar.activation(out=attnT[:, i, :], in_=pm[:],
                                             func=ACT.Exp, scale=INV_SQRT_HS)
                for h in pair:
                    pd = psum.tile([1, L], F32, tag="dn", bufs=1, name="pd")
                    for i in range(LC):
                        nc.tensor.matmul(pd[:], ones_r[:], atts[h][:, i, :],
                                         start=(i == 0), stop=(i == LC - 1))
                    recip = ap.tile([1, L], F32, tag="recip", bufs=2, name="recip")
                    nc.vector.reciprocal_approx_fast(out=recip[:], in_=pd[:])
                    rbc = ap.tile([P, L], F32, tag="rbc", bufs=2, name="rbc")
                    nc.gpsimd.partition_broadcast(rbc[:], recip[:])
                    rbcs[h] = rbc
                for h in pair:
                    po = pmm()
                    for i in range(LC):
                        nc.tensor.matmul(po[:], v_sb[:, i, ts(h, P)],
                                         atts[h][:, i, :],
                                         start=(i == 0), stop=(i == LC - 1))
                    nc.vector.tensor_tensor(out=OT[:, h, :], in0=po[:],
                                            in1=rbcs[h][:], op=OP.mult)

            # ---- proj + residual -> x1 ----
            for m in range(LC):
                pm = pmm()
                for k in range(CC):
                    nc.tensor.matmul(pm[:], OT[:, k, ts(m, P)], wp[:, k, :],
                                     start=(k == 0), stop=(k == CC - 1))
                nc.vector.tensor_tensor(out=x1[b][:, m, :], in0=x_t[:, m, :],
                                        in1=pm[:], op=OP.add)

        ln0 = attn_ln(0)
        attn_phase(0, *ln0)
        ln1 = attn_ln(1)          # b1's LN fills b0->b1 boundary idle
        gin_inputs[0] = gin_pre(0)
        attn_phase(1, *ln1)
        gin_inputs[1] = gin_pre(1)
        attn_stack.close()

        # ================= GIN main =================
        with ExitStack() as gin_stack:
            wB = gin_stack.enter_context(tc.tile_pool(name="wB", bufs=1))
            gp = gin_stack.enter_context(tc.tile_pool(name="gin", bufs=1))

            wgc = wB.tile([P, CC, HID], F32R)
            wgcn_r3 = wgcn_d.rearrange("(ko p) n -> p ko n", p=P).bitcast(F32R)
            for k in range(CC):
                nc.sync.dma_start(out=wgc[:, k, :], in_=wgcn_r3[:, k, :])
            wf1 = wB.tile([P, CC, HID], F32R)
            wfc1_r3 = wfc1_d.rearrange("(ko p) n -> p ko n", p=P).bitcast(F32R)
            for k in range(CC):
                nc.sync.dma_start(out=wf1[:, k, :], in_=wfc1_r3[:, k, :])
            wf2_b = wB.tile([P, HC, C], F32R)
            wfc2_r3 = wfc2_d.rearrange("(ko p) n -> p ko n", p=P).bitcast(F32R)
            for k in range(0, HC, 4):
                nc.sync.dma_start(out=wf2_b[:, k:k + 4, :], in_=wfc2_r3[:, k:k + 4, :])

            for b in range(NB):
                adj_b, adjT_b, xn2T, xc2_b = gin_inputs[b]

                # ---- u1T = (adj @ xn2)^T, u2T = (adjT @ xn2)^T ----
                # adj@(xn2@Wg1) = (adj@xn2)@Wg1: computing uT first saves
                # 32 N=512 matmuls vs materializing g = xn2@Wgcn.
                # u1T[c,l] = sum_m xn2hat[m,c]*adjT[m,l]; gamma2 folded into
                # the copyback (per-partition scalar in channel-major).
                u1T = gp.tile([P, CC, L], F32R, tag="u1T", name="u1T")
                u2T = gp.tile([P, CC, L], F32R, tag="u2T", name="u2T")
                for dst, rhs in ((u1T, adjT_b), (u2T, adj_b)):
                    for m in range(CC):
                        pm = pmm()
                        for k in range(LC):
                            nc.tensor.matmul(pm[:], xc2_b[:, k, ts(m, P)],
                                             rhs[:, k, :],
                                             start=(k == 0), stop=(k == LC - 1))
                        nc.vector.tensor_scalar(out=dst[:, m, :], in0=pm[:],
                                                scalar1=ln2g[:, m:m + 1],
                                                scalar2=None, op0=OP.mult)

                # ---- hT = relu(fc1 + [u1@Wg1; u2@Wg2])^T  (hid-major) ----
                hT_r = gp.tile([P, HC, L], F32R, tag="hT_r", name="hT_r")
                for mh in range(HC):
                    pm = pmm()
                    uT = u1T if mh < HC // 2 else u2T
                    for k in range(CC):
                        nc.tensor.matmul(pm[:], wgc[:, k, ts(mh, P)], uT[:, k, :],
                                         start=(k == 0), stop=False)
                    for k in range(CC):
                        nc.tensor.matmul(pm[:], wf1[:, k, ts(mh, P)], xn2T[:, k, :],
                                         start=False, stop=(k == CC - 1))
                    nc.scalar.activation(out=hT_r[:, mh, :], in_=pm[:], func=ACT.Relu)

                # ---- out = x1 + hT.T @ w_fc2 ----
                for m in range(LC):
                    pm = pmm()
                    for k in range(HC):
                        nc.tensor.matmul(pm[:], hT_r[:, k, ts(m, P)], wf2_b[:, k, :],
                                         start=(k == 0), stop=(k == HC - 1))
                    o_sb = gp.tile([P, C], F32, tag="o_sb", bufs=2, name="o_sb")
                    nc.vector.tensor_tensor(out=o_sb[:], in0=x1[b][:, m, :],
                                            in1=pm[:], op=OP.add)
                    nc.sync.dma_start(out=out_t3[b][:, m, :], in_=o_sb[:])


# ======================= SPMD wrapper =======================
import numpy as np

N_CORES = 8
_CACHE = {}


def _get_program():
    if "nc" not in _CACHE:
        from concourse import bacc
        nc = bacc.Bacc("TRN2", target_bir_lowering=False, debug=False,
                       num_devices=N_CORES)
        build_encoder_program(nc)
        nc.finalize()
        _CACHE["nc"] = nc
    return _CACHE["nc"]


def kernel(**inputs):
    """Full-input entry point: shards batch dim over 8 NeuronCores,
    runs the Bass program, gathers the full output."""
    from concourse.bass_utils import run_bass_kernel_spmd

    nc = _get_program()
    B = inputs["x"].shape[0]
    assert B == NB * N_CORES, f"expected B={NB * N_CORES}, got {B}"
    shared = {k: np.ascontiguousarray(np.asarray(v, np.float32))
              for k, v in inputs.items() if k not in ("x", "rel_pos", "adj")}
    in_maps = []
    for c in range(N_CORES):
        sl = slice(NB * c, NB * (c + 1))
        m = dict(shared)
        for k in ("x", "rel_pos", "adj"):
            m[k] = np.ascontiguousarray(np.asarray(inputs[k], np.float32)[sl])
        in_maps.append(m)
    res = run_bass_kernel_spmd(nc, in_maps, list(range(N_CORES)))
    return np.concatenate([res.results[c]["out"] for c in range(N_CORES)], axis=0)



# revision 32
# speedup vs baseline: 1.1021x; 1.1021x over previous
"""Bass/Tile kernel for nn_EncoderBlock (dense transformer w/ graph-masked
attention + GIN MLP). Per-core program: 2 batches, L=512, C=512, H=4, HS=128,
HID=2048. Data-parallel over batch across 8 cores, no collectives.

v2 design (vs v0 baseline at ~269us):
  - All matmuls bf16 (weights cast + LN-gamma folded on HOST; activations
    quantized on-chip). LN beta handled exactly: per-partition adds on
    channel-major copybacks, broadcast-row add for v, ACT bias for fc1,
    rank-1 adj-rowsum term fused into the u copyback (scalar_tensor_tensor).
  - All transposes moved off the PE onto the DMA crossbar
    (dma_start_transpose): relT/adjT loaded transposed straight from DRAM,
    xn1T/xn2T transposed SBUF->SBUF from the normalized activations.
  - Hop masks kept positive (0/1 in fp8e4, diagonal filled via
    affine_select) and applied as a DVE multiply on exp(score) instead of a
    -inf bias matmul. m2=aTa/m3=aaT via fp8 DoubleRow matmuls.
  - Softmax denominators: 3 DVE chunk-adds fold attnT to [P,L], then a
    single ones-vector matmul per head (4 instead of 16 PE ops).
  - Head stages software-pipelined S/D/A with independent GEMM blocks
    (other batch's QKV/masks, first GIN hT chunks) interleaved as PE
    fillers so the tensor engine never idles on the softmax chain.
  - Host pre-casts x/rel/adj/weights to bf16: input DMA drops to ~12MB
    total; GIN weights prefetched during attention on the same queue.
"""

import sys
for _p in ("/opt/trn_rl_repo", "/root/.axon_site/_ro/trn_rl_repo"):
    if _p not in sys.path:
        sys.path.append(_p)

from contextlib import ExitStack

import numpy as np
import ml_dtypes

import concourse.bass as bass
import concourse.tile as tile
from concourse import mybir
from concourse.bass import ts

F32 = mybir.dt.float32
BF16 = mybir.dt.bfloat16
FP8 = mybir.dt.float8e4
I32 = mybir.dt.int32
OP = mybir.AluOpType
ACT = mybir.ActivationFunctionType
DR = mybir.MatmulPerfMode.DoubleRow

P = 128
L = 512
C = 512
H = 4
HS = 128
HID = 2048
NB = 2          # batches per core
LC = L // P     # 4 token chunks
CC = C // P     # 4 channel chunks
HC = HID // P   # 16 hidden chunks
EPS = 1e-5
INV_SQRT_HS = 1.0 / (HS ** 0.5)
N_WARM = 20


def build_encoder_program(nc):
    """Emit the full 2-batch encoder program into `nc`."""
    def dram(name, shape, dt, kind):
        return nc.dram_tensor(name, shape, dt, kind=kind).ap()

    x_d = dram("x", [NB, L, C], BF16, "ExternalInput")
    rel_d = dram("rel", [NB, L, L], BF16, "ExternalInput")
    adj_d = dram("adj", [NB, L, L], BF16, "ExternalInput")
    wqkv_d = dram("wqkv", [C, 3 * C], BF16, "ExternalInput")
    wproj_d = dram("wproj", [C, C], BF16, "ExternalInput")
    wgcn_d = dram("wgcn", [C, HID], BF16, "ExternalInput")
    wfc1_d = dram("wfc1", [C, HID], BF16, "ExternalInput")
    wfc2_d = dram("wfc2", [HID, C], BF16, "ExternalInput")
    qkb_d = dram("qkb", [P, 2 * CC], F32, "ExternalInput")
    vbr_d = dram("vbr", [1, C], BF16, "ExternalInput")
    fc1b_d = dram("fc1b", [P, HC], F32, "ExternalInput")
    ln2b_d = dram("ln2b", [P, CC], F32, "ExternalInput")
    adjsum_d = dram("adjsum", [NB, 1, 2 * L], BF16, "ExternalInput")
    out_d = dram("out", [NB, L, C], F32, "ExternalOutput")

    x_t3 = [x_d[b].rearrange("(lo p) c -> p lo c", p=P) for b in range(NB)]
    rel_t3 = [rel_d[b].rearrange("(lo p) c -> p lo c", p=P) for b in range(NB)]
    adj_t3 = [adj_d[b].rearrange("(lo p) c -> p lo c", p=P) for b in range(NB)]
    out_t3 = [out_d[b].rearrange("(lo p) c -> p lo c", p=P) for b in range(NB)]

    with ExitStack() as top:
        tc = top.enter_context(tile.TileContext(nc))
        const = top.enter_context(tc.tile_pool(name="const", bufs=1))
        pool = top.enter_context(tc.tile_pool(name="main", bufs=1))
        psum = top.enter_context(tc.tile_pool(name="psum", bufs=1, space="PSUM"))

        def pmm():
            return psum.tile([P, 512], F32, tag="mm", bufs=5, name="pmm")

        # ================= input DMAs =================
        # scalar queue: small bias tensors (ready early, off the main stream)
        qkb = const.tile([P, 2 * CC], F32)
        nc.scalar.dma_start(out=qkb[:], in_=qkb_d[:, :])
        fc1b = const.tile([P, HC], F32)
        nc.scalar.dma_start(out=fc1b[:], in_=fc1b_d[:, :])
        ln2b = const.tile([P, CC], F32)
        nc.scalar.dma_start(out=ln2b[:], in_=ln2b_d[:, :])
        vbr = const.tile([1, C], BF16)
        nc.scalar.dma_start(out=vbr[:], in_=vbr_d[:, :])
        adjsum_rows = []
        for b in range(NB):
            r = const.tile([1, 2 * L], BF16, name=f"adjsum{b}")
            nc.scalar.dma_start(out=r[:], in_=adjsum_d[b])
            adjsum_rows.append(r)

        # sync queue: the big input stream, in consumption order
        x_t = [pool.tile([P, LC, C], BF16, tag="x_t", bufs=2, name="x_t")
               for _ in range(NB)]
        relx = {}   # (b, transposed?) -> [P, LC, L] bf16 tiles

        def dma_x(b):
            for i in range(LC):
                nc.sync.dma_start(out=x_t[b][:, i, :], in_=x_t3[b][:, i, :])

        def dma_rel(b):
            # one tag, bufs=2: rel/relT rotate; batch 1's DMA is emitted
            # after batch 0's mask readers so the WAR wait is well defined
            r = pool.tile([P, LC, L], BF16, tag="relx", bufs=2, name="rel")
            for i in range(LC):
                nc.sync.dma_start(out=r[:, i, :], in_=rel_t3[b][:, i, :])
            relx[(b, 0)] = r
            rt = pool.tile([P, LC, L], BF16, tag="relx", bufs=2, name="relT")
            for j in range(LC):
                nc.sync.dma_start_transpose(out=rt[:, j, :],
                                            in_=rel_d[b][:, ts(j, P)])
            relx[(b, 1)] = rt

        dma_x(0)
        dma_rel(0)
        wA_cm = tc.tile_pool(name="wA", bufs=1)
        wA = wA_cm.__enter__()
        wq = wA.tile([P, CC, 3 * C], BF16, name="wq")
        nc.sync.dma_start(out=wq[:],
                          in_=wqkv_d.rearrange("(ko p) n -> p ko n", p=P))
        wp = wA.tile([P, CC, C], BF16, name="wp")
        nc.sync.dma_start(out=wp[:],
                          in_=wproj_d.rearrange("(ko p) n -> p ko n", p=P))
        dma_x(1)

        adj_sb = [None] * NB
        adjT_sb = [None] * NB

        def dma_adj(b):
            # bufs=1: batch 1's DMA is emitted after batch 0's u_block
            # readers, so the tag-rotation WAR wait is well defined
            a = pool.tile([P, LC, L], BF16, tag="adj", bufs=1, name="adj")
            for i in range(LC):
                nc.sync.dma_start(out=a[:, i, :], in_=adj_t3[b][:, i, :])
            at = pool.tile([P, LC, L], BF16, tag="adjT", bufs=1, name="adjT")
            for j in range(LC):
                nc.sync.dma_start_transpose(out=at[:, j, :],
                                            in_=adj_d[b][:, ts(j, P)])
            adj_sb[b], adjT_sb[b] = a, at

        def dma_gin_weights():
            w1 = pool.tile([P, CC, HID], BF16, name="wgc")
            nc.sync.dma_start(out=w1[:],
                              in_=wgcn_d.rearrange("(ko p) n -> p ko n", p=P))
            w2 = pool.tile([P, CC, HID], BF16, name="wf1")
            nc.sync.dma_start(out=w2[:],
                              in_=wfc1_d.rearrange("(ko p) n -> p ko n", p=P))
            return w1, w2

        # ================= constants =================
        neg5 = const.tile([P, 1], F32)
        nc.vector.memset(neg5[:], -5.0)
        ones_b = const.tile([P, 1], BF16)
        nc.vector.memset(ones_b[:], 1.0)
        magic4 = const.tile([P, LC], I32)
        nc.vector.memset(magic4[:], 0x5F3759DF)
        warm = const.tile([P, 512], BF16)
        nc.vector.memset(warm[:], 0.0)
        vb = const.tile([P, C], BF16)
        nc.gpsimd.partition_broadcast(vb[:], vbr[:])
        adjsum_bc = []
        for b in range(NB):
            t = const.tile([P, 2 * L], BF16, name=f"adjsum_bc{b}")
            nc.gpsimd.partition_broadcast(t[:], adjsum_rows[b][:])
            adjsum_bc.append(t)

        # PE warmup: keep the tensor engine streaming during the input DMA
        # head so the clock ramps to the high p-state before real work.
        for _ in range(N_WARM):
            pw = pmm()
            nc.tensor.matmul(pw[:], warm[:, 0:P], warm[:], start=True, stop=True)

        # ================= helpers =================
        def layer_norm_T(xin, tag, keep_xc=False):
            """xin [P, LC, C] bf16 -> xnT [P, CC, L] bf16 (no gamma/beta;
            gamma is folded into the consuming weights on the host, beta is
            re-added downstream). Optionally also keep the normalized
            token-major copy (for the GIN u matmuls)."""
            xnT = pool.tile([P, CC, L], BF16, tag=f"xnT_{tag}",
                            bufs=(2 if tag == "2" else 1), name=f"xnT{tag}")
            xc_full = None
            if keep_xc:
                xc_full = pool.tile([P, LC, C], BF16, tag="xc2", bufs=1,
                                    name="xc2")
            mu4 = pool.tile([P, LC], F32, tag="ln_mu4", bufs=2, name="mu4")
            s4 = pool.tile([P, LC], F32, tag="ln_s4", bufs=2, name="s4")
            for i in range(LC):
                st6 = pool.tile([P, 6], F32, tag="ln_st6", bufs=2, name="st6")
                nc.vector.bn_stats(out=st6[:], in_=xin[:, i, :])
                mv = pool.tile([P, 2], F32, tag="ln_mv", bufs=2, name="mv")
                nc.vector.bn_aggr(out=mv[:], in_=st6[:])
                nc.vector.tensor_copy(out=mu4[:, i:i + 1], in_=mv[:, 0:1])
                nc.vector.tensor_scalar(out=s4[:, i:i + 1], in0=mv[:, 1:2],
                                        scalar1=EPS, scalar2=None, op0=OP.add)
            # istd = rsqrt(var+eps): Quake seed + 2 Newton steps (DVE)
            y4 = pool.tile([P, LC], F32, tag="ln_y4", bufs=2, name="y4")
            t4 = pool.tile([P, LC], F32, tag="ln_t4", bufs=2, name="t4")
            nc.vector.tensor_scalar(out=t4[:].bitcast(I32), in0=s4[:].bitcast(I32),
                                    scalar1=1, scalar2=None,
                                    op0=OP.arith_shift_right)
            nc.vector.tensor_tensor(out=y4[:].bitcast(I32), in0=magic4[:],
                                    in1=t4[:].bitcast(I32), op=OP.subtract)
            for _ in range(2):
                nc.vector.tensor_tensor(out=t4[:], in0=y4[:], in1=y4[:], op=OP.mult)
                nc.vector.tensor_tensor(out=t4[:], in0=t4[:], in1=s4[:], op=OP.mult)
                nc.vector.tensor_scalar(out=t4[:], in0=t4[:], scalar1=-0.5,
                                        scalar2=1.5, op0=OP.mult, op1=OP.add)
                nc.vector.tensor_tensor(out=y4[:], in0=y4[:], in1=t4[:], op=OP.mult)
            for i in range(LC):
                if keep_xc:
                    xc = xc_full[:, i, :]
                else:
                    xcs = pool.tile([P, C], BF16, tag="xcstage", bufs=2,
                                    name="xcstage")
                    xc = xcs[:]
                nc.vector.tensor_scalar(out=xc, in0=xin[:, i, :],
                                        scalar1=mu4[:, i:i + 1],
                                        scalar2=y4[:, i:i + 1],
                                        op0=OP.subtract, op1=OP.mult)
                nc.scalar.dma_start_transpose(out=xnT[:, :, ts(i, P)], in_=xc)
            return xnT, xc_full

        def fill_diag(ap_2d, m, val):
            nc.gpsimd.affine_select(out=ap_2d, in_=ap_2d,
                                    compare_op=OP.not_equal, fill=val,
                                    base=P * m, pattern=[[-1, L]],
                                    channel_multiplier=1)

        # ---- hop masks (fp8 0/1, diag filled). masks[b][h] for heads ----
        masks = [[None] * H for _ in range(NB)]

        def mask_base(b):
            """a / aT fp8 tiles (raw, no diag) from rel / relT."""
            raw = []
            for t in range(2):
                a8 = pool.tile([P, LC, L], FP8, tag=f"a8_{t}", bufs=2,
                               name=f"a8_{t}")
                for i in range(LC):
                    tabs = pool.tile([P, L], BF16, tag="tabs", bufs=1,
                                     name="tabs")
                    nc.scalar.activation(out=tabs[:], in_=relx[(b, t)][:, i, :],
                                         func=ACT.Abs, bias=neg5[:], scale=1.0)
                    nc.vector.tensor_scalar(out=a8[:, i, :], in0=tabs[:],
                                            scalar1=4.0, scalar2=None,
                                            op0=OP.is_equal)
                raw.append(a8)
            return raw  # [a (rel-derived), aT (relT-derived)]

        def mask_mm(b, raw):
            """m2=aTa, m3=aaT via fp8 DoubleRow; then diag-fill all four.
            Emits 16 PE matmuls. Head order: scoresT chunks are [lk, lq], so
            head0 (mask a) uses the relT-derived tile and head1 the
            rel-derived one; m2/m3 are symmetric."""
            a8, aT8 = raw
            # bufs=1: batch 1's mask_mm is emitted after batch 0's S-stage
            # readers of m2/m3
            for idx, src in ((2, a8), (3, aT8)):
                cm = pool.tile([P, LC, L], FP8, tag=f"m{idx}", bufs=1,
                               name=f"m{idx}")
                for m in range(LC):
                    pm = pmm()
                    for k in range(LC // 2):
                        nc.tensor.matmul(pm[:],
                                         src[:, 2 * k:2 * k + 2, ts(m, P)],
                                         src[:, 2 * k:2 * k + 2, :],
                                         start=(k == 0), stop=(k == 1),
                                         perf_mode=DR)
                    nc.vector.tensor_scalar(out=cm[:, m, :], in0=pm[:],
                                            scalar1=0.5, scalar2=None,
                                            op0=OP.is_ge)
                    fill_diag(cm[:, m, :], m, 1.0)
                masks[b][idx] = cm
            for i in range(LC):
                fill_diag(aT8[:, i, :], i, 1.0)
                fill_diag(a8[:, i, :], i, 1.0)
            masks[b][0], masks[b][1] = aT8, a8

        # ---- QKV ----
        qT = [None] * NB
        kT = [None] * NB
        v_sb = [None] * NB

        def qk_block(b, xnT, dst_idx, mc_range):
            """channel-major q/k chunks; dst_idx 0=q, 1=k."""
            dst = qT if dst_idx == 0 else kT
            if dst[b] is None:
                dst[b] = pool.tile([P, CC, L], BF16, tag=f"qk{dst_idx}",
                                   bufs=2, name=f"qk{dst_idx}")
            off = dst_idx * C
            for m in mc_range:
                pm = pmm()
                for k in range(CC):
                    nc.tensor.matmul(pm[:], wq[:, k, off + m * P:off + (m + 1) * P],
                                     xnT[:, k, :],
                                     start=(k == 0), stop=(k == CC - 1))
                nc.vector.tensor_scalar(out=dst[b][:, m, :], in0=pm[:],
                                        scalar1=qkb[:, 4 * dst_idx + m:
                                                    4 * dst_idx + m + 1],
                                        scalar2=None, op0=OP.add)

        def v_block(b, xnT, mt_range):
            # bufs=1: batch 1's v is computed after batch 0's A-stages
            if v_sb[b] is None:
                v_sb[b] = pool.tile([P, LC, C], BF16, tag="v_sb", bufs=1,
                                    name="v_sb")
            for m in mt_range:
                pm = pmm()
                for k in range(CC):
                    nc.tensor.matmul(pm[:], xnT[:, k, ts(m, P)],
                                     wq[:, k, 2 * C:3 * C],
                                     start=(k == 0), stop=(k == CC - 1))
                nc.vector.tensor_tensor(out=v_sb[b][:, m, :], in0=pm[:],
                                        in1=vb[:], op=OP.add)

        # ---- attention head stages ----
        def S(b, h, atts):
            """scores + exp + mask-mult for head h -> attnT tile."""
            attnT = pool.tile([P, LC, L], BF16, tag="attnT", bufs=2,
                              name="attnT")
            atts[h] = attnT
            for i in range(LC):
                pm = pmm()
                nc.tensor.matmul(pm[:], kT[b][:, h, ts(i, P)], qT[b][:, h, :],
                                 start=True, stop=True)
                nc.scalar.activation(out=attnT[:, i, :], in_=pm[:],
                                     func=ACT.Exp, scale=INV_SQRT_HS)
                nc.vector.tensor_tensor(out=attnT[:, i, :], in0=attnT[:, i, :],
                                        in1=masks[b][h][:, i, :], op=OP.mult)

        def D(b, h, atts, rbcs):
            """denominator: fold chunks on DVE, one ones-matmul, recip, bcast."""
            at = atts[h]
            fold = pool.tile([P, L], BF16, tag="fold", bufs=2, name="fold")
            nc.vector.tensor_tensor(out=fold[:], in0=at[:, 0, :], in1=at[:, 1, :],
                                    op=OP.add)
            nc.vector.tensor_tensor(out=fold[:], in0=fold[:], in1=at[:, 2, :],
                                    op=OP.add)
            nc.vector.tensor_tensor(out=fold[:], in0=fold[:], in1=at[:, 3, :],
                                    op=OP.add)
            pd = psum.tile([1, L], F32, tag="dn", bufs=2, name="pd")
            nc.tensor.matmul(pd[:], ones_b[:], fold[:], start=True, stop=True)
            recip = pool.tile([1, L], F32, tag="recip", bufs=2, name="recip")
            nc.vector.reciprocal_approx_fast(out=recip[:], in_=pd[:])
            rbc = pool.tile([P, L], F32, tag="rbc", bufs=2, name="rbc")
            nc.gpsimd.partition_broadcast(rbc[:], recip[:])
            rbcs[h] = rbc

        def A(b, h, atts, rbcs, OT):
            po = pmm()
            for i in range(LC):
                nc.tensor.matmul(po[:], v_sb[b][:, i, ts(h, P)],
                                 atts[h][:, i, :],
                                 start=(i == 0), stop=(i == LC - 1))
            nc.vector.tensor_tensor(out=OT[:, h, :], in0=po[:],
                                    in1=rbcs[h][:], op=OP.mult)

        x1 = [None] * NB

        def proj(b, x_tile, OT):
            x1[b] = pool.tile([P, LC, C], BF16, tag="x1", bufs=2, name="x1")
            for m in range(LC):
                pm = pmm()
                for k in range(CC):
                    nc.tensor.matmul(pm[:], OT[:, k, ts(m, P)], wp[:, k, :],
                                     start=(k == 0), stop=(k == CC - 1))
                nc.vector.tensor_tensor(out=x1[b][:, m, :], in0=x_tile[:, m, :],
                                        in1=pm[:], op=OP.add)

        # ---- GIN ----
        u1T = [None] * NB
        u2T = [None] * NB

        def u_block(b, xc2, uidx, mc_range):
            """uT = ((adj|adjT) @ xn2)^T with the LN2-beta rank-1 term fused
            into the copyback: u += adj_rowsum[l] * beta2[c]."""
            lst = u1T if uidx == 0 else u2T
            # bufs=1: batch 1's u is emitted after batch 0's hT readers
            if lst[b] is None:
                lst[b] = pool.tile([P, CC, L], BF16, tag=f"u{uidx}", bufs=1,
                                   name=f"u{uidx}")
            rhs = adjT_sb[b] if uidx == 0 else adj_sb[b]
            for m in mc_range:
                pm = pmm()
                for k in range(LC):
                    nc.tensor.matmul(pm[:], xc2[:, k, ts(m, P)], rhs[:, k, :],
                                     start=(k == 0), stop=(k == LC - 1))
                nc.vector.scalar_tensor_tensor(out=lst[b][:, m, :],
                                               in0=adjsum_bc[b][:, ts(uidx, L)],
                                               scalar=ln2b[:, m:m + 1],
                                               in1=pm[:],
                                               op0=OP.mult, op1=OP.add)

        hT = [None] * NB

        def hT_block(b, xn2T, mh_range):
            if hT[b] is None:
                hT[b] = pool.tile([P, HC, L], BF16, tag="hT", bufs=1, name="hT")
            for mh in mh_range:
                pm = pmm()
                uT = u1T[b] if mh < HC // 2 else u2T[b]
                for k in range(CC):
                    nc.tensor.matmul(pm[:], wgc[:, k, ts(mh, P)], uT[:, k, :],
                                     start=(k == 0), stop=False)
                for k in range(CC):
                    nc.tensor.matmul(pm[:], wf1[:, k, ts(mh, P)], xn2T[:, k, :],
                                     start=False, stop=(k == CC - 1))
                nc.scalar.activation(out=hT[b][:, mh, :], in_=pm[:],
                                     func=ACT.Relu, bias=fc1b[:, mh:mh + 1],
                                     scale=1.0)

        def fc2_block(b, mt_range, wf2):
            for m in mt_range:
                pm = pmm()
                for k in range(HC):
                    nc.tensor.matmul(pm[:], hT[b][:, k, ts(m, P)], wf2[:, k, :],
                                     start=(k == 0), stop=(k == HC - 1))
                o_sb = pool.tile([P, C], F32, tag="o_sb", bufs=2, name="o_sb")
                nc.vector.tensor_tensor(out=o_sb[:], in0=x1[b][:, m, :],
                                        in1=pm[:], op=OP.add)
                nc.sync.dma_start(out=out_t3[b][:, m, :], in_=o_sb[:])

        # ================= schedule =================
        # batch 0 front: LN1 + masks + QKV
        xn1T_0, _ = layer_norm_T(x_t[0], "1")
        raw0 = mask_base(0)
        # late-emitted input DMAs: queue position is after wq/wp/x1, and the
        # relx tag-rotation WAR (bufs=2) sees batch 0's readers above
        dma_rel(1)
        dma_adj(0)
        wgc, wf1 = dma_gin_weights()
        mask_mm(0, raw0)
        for m in range(CC):
            qk_block(0, xn1T_0, 0, [m])
            qk_block(0, xn1T_0, 1, [m])
        v_block(0, xn1T_0, range(LC))

        # batch 1 LN + mask DVE prep (PE-free) before batch 0 heads
        xn1T_1, _ = layer_norm_T(x_t[1], "1")
        raw1 = mask_base(1)

        # batch 0 heads; fillers: b1 QKV, b1 mask matmuls (the latter must
        # come after S(0,2)/S(0,3) since m2/m3 are bufs=1)
        atts, rbcs = {}, {}
        OT0 = pool.tile([P, H, L], BF16, tag="OT", bufs=1, name="OT")
        S(0, 0, atts)
        S(0, 1, atts)
        D(0, 0, atts, rbcs)
        qk_block(1, xn1T_1, 0, range(2))      # F1: 8 mm
        A(0, 0, atts, rbcs, OT0)
        D(0, 1, atts, rbcs)
        S(0, 2, atts)
        qk_block(1, xn1T_1, 0, range(2, CC))  # F2: 8 mm
        A(0, 1, atts, rbcs, OT0)
        D(0, 2, atts, rbcs)
        S(0, 3, atts)
        qk_block(1, xn1T_1, 1, range(2))      # F3: 8 mm
        A(0, 2, atts, rbcs, OT0)
        D(0, 3, atts, rbcs)
        mask_mm(1, raw1)                      # F4: 16 mm
        A(0, 3, atts, rbcs, OT0)
        qk_block(1, xn1T_1, 1, range(2, CC))  # F5: 8 mm (covers OT0 drain)
        v_block(1, xn1T_1, range(LC))         # 16 mm; v bufs=1 safe here
        proj(0, x_t[0], OT0)

        # batch 0 LN2 (DVE + DMA transposes, runs under b1 heads)
        xn2T_0, xc2_0 = layer_norm_T(x1[0], "2", keep_xc=True)

        # batch 1 heads; fillers: b0 u matmuls, first b0 hT chunks
        atts, rbcs = {}, {}
        OT1 = pool.tile([P, H, L], BF16, tag="OT", bufs=1, name="OT")
        S(1, 0, atts)
        S(1, 1, atts)
        D(1, 0, atts, rbcs)
        u_block(0, xc2_0, 0, range(CC))       # F1: 16 mm
        A(1, 0, atts, rbcs, OT1)
        D(1, 1, atts, rbcs)
        S(1, 2, atts)
        u_block(0, xc2_0, 1, range(CC))       # F2: 16 mm
        A(1, 1, atts, rbcs, OT1)
        D(1, 2, atts, rbcs)
        S(1, 3, atts)
        # adj b1 DMA: emitted after all adj b0 readers (u_block above)
        dma_adj(1)
        hT_block(0, xn2T_0, range(0, 2))      # F3: 16 mm
        A(1, 2, atts, rbcs, OT1)
        D(1, 3, atts, rbcs)
        hT_block(0, xn2T_0, range(2, 4))      # F4: 16 mm
        A(1, 3, atts, rbcs, OT1)
        hT_block(0, xn2T_0, range(4, 8))      # F5: 32 mm (covers OT1 drain)
        proj(1, x_t[1], OT1)
        # attention weights are dead now; close their pool and stream wf2
        # into the freed region (needed ~25us later by fc2_block(0))
        wA_cm.__exit__(None, None, None)
        with tc.tile_pool(name="wB", bufs=1) as wB:
            wf2 = wB.tile([P, HC, C], BF16, name="wf2")
            nc.sync.dma_start(out=wf2[:],
                              in_=wfc2_d.rearrange("(ko p) n -> p ko n", p=P))

            # batch 1 LN2; PE keeps going on b0 GIN
            xn2T_1, xc2_1 = layer_norm_T(x1[1], "2", keep_xc=True)
            hT_block(0, xn2T_0, range(8, HC))
            u_block(1, xc2_1, 0, range(CC))
            u_block(1, xc2_1, 1, range(CC))
            fc2_block(0, range(LC), wf2)
            hT_block(1, xn2T_1, range(HC))
            fc2_block(1, range(LC), wf2)


# ======================= SPMD wrapper =======================
N_CORES = 8
_CACHE = {}


def _get_program():
    if "nc" not in _CACHE:
        from concourse import bacc
        nc = bacc.Bacc("TRN2", target_bir_lowering=False, debug=False,
                       num_devices=N_CORES)
        build_encoder_program(nc)
        nc.finalize()
        _CACHE["nc"] = nc
    return _CACHE["nc"]


def prep_in_maps(inputs):
    """Host-side prep: cast to bf16, fold LN gammas into the consuming
    weights, precompute LN-beta bias rows and adj row/col sums."""
    BF = ml_dtypes.bfloat16
    f32 = np.float32
    g = lambda k: np.asarray(inputs[k], f32)
    x, rel, adj = g("x"), g("rel_pos"), g("adj")
    g1, b1 = g("ln1_g"), g("ln1_b")
    g2, b2 = g("ln2_g"), g("ln2_b")
    wqkv, wproj = g("w_qkv"), g("w_proj")
    wfc1, wgcn, wfc2 = g("w_fc1"), g("w_gcn"), g("w_fc2")

    qkvb = b1 @ wqkv                      # [3C]
    shared = {
        "wqkv": np.ascontiguousarray((g1[:, None] * wqkv).astype(BF)),
        "wproj": np.ascontiguousarray(wproj.astype(BF)),
        "wgcn": np.ascontiguousarray((g2[:, None] * wgcn).astype(BF)),
        "wfc1": np.ascontiguousarray((g2[:, None] * wfc1).astype(BF)),
        "wfc2": np.ascontiguousarray(wfc2.astype(BF)),
        "qkb": np.ascontiguousarray(qkvb[:2 * C].reshape(2 * CC, P).T.astype(f32)),
        "vbr": np.ascontiguousarray(qkvb[None, 2 * C:].astype(BF)),
        "fc1b": np.ascontiguousarray((b2 @ wfc1).reshape(HC, P).T.astype(f32)),
        "ln2b": np.ascontiguousarray(b2.reshape(CC, P).T.astype(f32)),
    }
    in_maps = []
    for c in range(N_CORES):
        sl = slice(NB * c, NB * (c + 1))
        xs, rs, ads = x[sl], rel[sl], adj[sl]
        m = dict(shared)
        m["x"] = np.ascontiguousarray(xs.astype(BF))
        m["rel"] = np.ascontiguousarray(rs.astype(BF))
        m["adj"] = np.ascontiguousarray(ads.astype(BF))
        m["adjsum"] = np.ascontiguousarray(
            np.stack([ads.sum(2), ads.sum(1)], axis=1)
            .reshape(NB, 1, 2 * L).astype(BF))
        in_maps.append(m)
    return in_maps


def kernel(**inputs):
    """Full-input entry point: shards batch dim over 8 NeuronCores,
    runs the Bass program, gathers the full output."""
    from concourse.bass_utils import run_bass_kernel_spmd

    nc = _get_program()
    B = inputs["x"].shape[0]
    assert B == NB * N_CORES, f"expected B={NB * N_CORES}, got {B}"
    in_maps = prep_in_maps(inputs)
    res = run_bass_kernel_spmd(nc, in_maps, list(range(N_CORES)))
    return np.concatenate([res.results[c]["out"] for c in range(N_CORES)], axis=0)


# revision 51
# speedup vs baseline: 1.2110x; 1.0988x over previous
"""Bass/Tile kernel for nn_EncoderBlock (dense transformer w/ graph-masked
attention + GIN MLP). Per-core program: 2 batches, L=512, C=512, H=4, HS=128,
HID=2048. Data-parallel over batch across 8 cores, no collectives.

v2 design (vs v0 baseline at ~269us):
  - All matmuls bf16 (weights cast + LN-gamma folded on HOST; activations
    quantized on-chip). LN beta handled exactly: per-partition adds on
    channel-major copybacks, broadcast-row add for v, ACT bias for fc1,
    rank-1 adj-rowsum term fused into the u copyback (scalar_tensor_tensor).
  - All transposes moved off the PE onto the DMA crossbar
    (dma_start_transpose): relT/adjT loaded transposed straight from DRAM,
    xn1T/xn2T transposed SBUF->SBUF from the normalized activations.
  - Hop masks kept positive (0/1 in fp8e4, diagonal filled via
    affine_select) and applied as a DVE multiply on exp(score) instead of a
    -inf bias matmul. m2=aTa/m3=aaT via fp8 DoubleRow matmuls.
  - Softmax denominators: 3 DVE chunk-adds fold attnT to [P,L], then a
    single ones-vector matmul per head (4 instead of 16 PE ops).
  - Head stages software-pipelined S/D/A with independent GEMM blocks
    (other batch's QKV/masks, first GIN hT chunks) interleaved as PE
    fillers so the tensor engine never idles on the softmax chain.
  - Host pre-casts x/rel/adj/weights to bf16: input DMA drops to ~12MB
    total; GIN weights prefetched during attention on the same queue.
"""

import sys
for _p in ("/opt/trn_rl_repo", "/root/.axon_site/_ro/trn_rl_repo"):
    if _p not in sys.path:
        sys.path.append(_p)

from contextlib import ExitStack

import numpy as np
import ml_dtypes

import concourse.bass as bass
import concourse.tile as tile
from concourse import mybir
from concourse.bass import ts
from concourse.masks import make_identity

F32 = mybir.dt.float32
BF16 = mybir.dt.bfloat16
FP8 = mybir.dt.float8e4
I32 = mybir.dt.int32
OP = mybir.AluOpType
ACT = mybir.ActivationFunctionType
DR = mybir.MatmulPerfMode.DoubleRow

P = 128
L = 512
C = 512
H = 4
HS = 128
HID = 2048
NB = 2          # batches per core
LC = L // P     # 4 token chunks
CC = C // P     # 4 channel chunks
HC = HID // P   # 16 hidden chunks
EPS = 1e-5
INV_SQRT_HS = 1.0 / (HS ** 0.5)
N_WARM = 8


def build_encoder_program(nc):
    """Emit the full 2-batch encoder program into `nc`."""
    def dram(name, shape, dt, kind):
        return nc.dram_tensor(name, shape, dt, kind=kind).ap()

    x_d = dram("x", [NB, L, C], BF16, "ExternalInput")
    rel_d = dram("rel", [NB, L, L], BF16, "ExternalInput")
    adj_d = dram("adj", [NB, L, L], BF16, "ExternalInput")
    wqkv_d = dram("wqkv", [C, 3 * C], BF16, "ExternalInput")
    wproj_d = dram("wproj", [C, C], BF16, "ExternalInput")
    wgcn_d = dram("wgcn", [C, HID], BF16, "ExternalInput")
    wfc1_d = dram("wfc1", [C, HID], BF16, "ExternalInput")
    wfc2_d = dram("wfc2", [HID, C], BF16, "ExternalInput")
    qkb_d = dram("qkb", [P, 2 * CC], F32, "ExternalInput")
    vbr_d = dram("vbr", [1, C], BF16, "ExternalInput")
    fc1b_d = dram("fc1b", [P, HC], F32, "ExternalInput")
    ln2b_d = dram("ln2b", [P, CC], F32, "ExternalInput")
    adjsum_d = dram("adjsum", [NB, 1, 2 * L], BF16, "ExternalInput")
    out_d = dram("out", [NB, L, C], F32, "ExternalOutput")

    x_t3 = [x_d[b].rearrange("(lo p) c -> p lo c", p=P) for b in range(NB)]
    rel_t3 = [rel_d[b].rearrange("(lo p) c -> p lo c", p=P) for b in range(NB)]
    adj_t3 = [adj_d[b].rearrange("(lo p) c -> p lo c", p=P) for b in range(NB)]
    out_t3 = [out_d[b].rearrange("(lo p) c -> p lo c", p=P) for b in range(NB)]

    with ExitStack() as top:
        tc = top.enter_context(tile.TileContext(nc))
        const = top.enter_context(tc.tile_pool(name="const", bufs=1))
        pool = top.enter_context(tc.tile_pool(name="main", bufs=1))
        psum = top.enter_context(tc.tile_pool(name="psum", bufs=1, space="PSUM"))

        def pmm():
            return psum.tile([P, 512], F32, tag="mm", bufs=5, name="pmm")

        def transpose_group(srcs, out_view):
            """4 PE transposes into one PSUM tile, single DVE copyback.
            srcs: list of 4 [P,128] bf16 APs; out_view: [P,4,128] AP."""
            pt = psum.tile([P, 4, P], BF16, tag="tp", bufs=2, name="ptp")
            for j in range(4):
                nc.tensor.transpose(pt[:, j, :], srcs[j], ident_b[:])
            nc.vector.tensor_copy(out=out_view, in_=pt[:])

        # ================= input DMAs =================
        # scalar queue: small bias tensors (ready early, off the main stream)
        qkb = const.tile([P, 2 * CC], F32)
        nc.scalar.dma_start(out=qkb[:], in_=qkb_d[:, :])
        fc1b = const.tile([P, HC], F32)
        nc.scalar.dma_start(out=fc1b[:], in_=fc1b_d[:, :])
        ln2b = const.tile([P, CC], F32)
        nc.scalar.dma_start(out=ln2b[:], in_=ln2b_d[:, :])
        vbr = const.tile([1, C], BF16)
        nc.scalar.dma_start(out=vbr[:], in_=vbr_d[:, :])
        adjsum_rows = []
        for b in range(NB):
            r = const.tile([1, 2 * L], BF16, name=f"adjsum{b}")
            nc.scalar.dma_start(out=r[:], in_=adjsum_d[b])
            adjsum_rows.append(r)

        # sync queue: the big input stream, in consumption order
        x_t = [pool.tile([P, LC, C], BF16, tag="x_t", bufs=2, name="x_t")
               for _ in range(NB)]
        relx = {}   # (b, transposed?) -> [P, LC, L] bf16 tiles

        def dma_x(b):
            for i in range(LC):
                nc.sync.dma_start(out=x_t[b][:, i, :], in_=x_t3[b][:, i, :])

        def dma_rel(b):
            # bufs=2: batch 1's DMA is emitted after batch 0's mask readers
            r = pool.tile([P, LC, L], BF16, tag="relx", bufs=2, name="rel")
            for i in range(LC):
                nc.sync.dma_start(out=r[:, i, :], in_=rel_t3[b][:, i, :])
            relx[b] = r

        dma_x(0)
        dma_rel(0)
        wA_cm = tc.tile_pool(name="wA", bufs=1)
        wA = wA_cm.__enter__()
        wq = wA.tile([P, CC, 3 * C], BF16, name="wq")
        nc.sync.dma_start(out=wq[:],
                          in_=wqkv_d.rearrange("(ko p) n -> p ko n", p=P))
        wp = wA.tile([P, CC, C], BF16, name="wp")
        nc.sync.dma_start(out=wp[:],
                          in_=wproj_d.rearrange("(ko p) n -> p ko n", p=P))
        dma_x(1)

        adj_sb = [None] * NB
        adjT_sb = [None] * NB

        def dma_adj(b):
            # bufs=1: batch 1's DMA is emitted after batch 0's u_block
            # readers, so the tag-rotation WAR wait is well defined
            a = pool.tile([P, LC, L], BF16, tag="adj", bufs=1, name="adj")
            for i in range(LC):
                nc.sync.dma_start(out=a[:, i, :], in_=adj_t3[b][:, i, :])
            adj_sb[b] = a

        def adjT_transpose(b):
            """PE-transpose adj -> adjT."""
            at = pool.tile([P, LC, L], BF16, tag="adjT", bufs=1, name="adjT")
            for i in range(LC):
                transpose_group([adj_sb[b][:, i, ts(j, P)] for j in range(LC)],
                                at[:, :, ts(i, P)])
            adjT_sb[b] = at

        def dma_gin_weights():
            w1 = pool.tile([P, CC, HID], BF16, name="wgc")
            nc.sync.dma_start(out=w1[:],
                              in_=wgcn_d.rearrange("(ko p) n -> p ko n", p=P))
            w2 = pool.tile([P, CC, HID], BF16, name="wf1")
            nc.sync.dma_start(out=w2[:],
                              in_=wfc1_d.rearrange("(ko p) n -> p ko n", p=P))
            return w1, w2

        # ================= constants =================
        ident_f = const.tile([P, P], F32)
        make_identity(nc, ident_f[:])
        ident_b = const.tile([P, P], BF16)
        nc.vector.tensor_copy(out=ident_b[:], in_=ident_f[:])
        neg5 = const.tile([P, 1], F32)
        nc.vector.memset(neg5[:], -5.0)
        ones_b = const.tile([P, 1], BF16)
        nc.vector.memset(ones_b[:], 1.0)
        magic4 = const.tile([P, LC], I32)
        nc.vector.memset(magic4[:], 0x5F3759DF)
        warm = const.tile([P, 512], BF16)
        nc.vector.memset(warm[:], 0.0)
        vb = const.tile([P, C], BF16)
        nc.gpsimd.partition_broadcast(vb[:], vbr[:])
        adjsum_bc = []
        for b in range(NB):
            t = const.tile([P, 2 * L], BF16, name=f"adjsum_bc{b}")
            nc.gpsimd.partition_broadcast(t[:], adjsum_rows[b][:])
            adjsum_bc.append(t)

        # PE warmup: keep the tensor engine streaming during the input DMA
        # head so the clock ramps to the high p-state before real work.
        for _ in range(N_WARM):
            pw = pmm()
            nc.tensor.matmul(pw[:], warm[:, 0:P], warm[:], start=True, stop=True)

        # ================= helpers =================
        def ln_stats(xin):
            """bn stats + rsqrt(var+eps) via Quake seed + 2 Newton steps.
            Pure DVE, no PE work. Returns (mu4, y4=istd)."""
            mu4 = pool.tile([P, LC], F32, tag="ln_mu4", bufs=2, name="mu4")
            s4 = pool.tile([P, LC], F32, tag="ln_s4", bufs=2, name="s4")
            for i in range(LC):
                st6 = pool.tile([P, 6], F32, tag="ln_st6", bufs=2, name="st6")
                nc.vector.bn_stats(out=st6[:], in_=xin[:, i, :])
                mv = pool.tile([P, 2], F32, tag="ln_mv", bufs=2, name="mv")
                nc.vector.bn_aggr(out=mv[:], in_=st6[:])
                nc.vector.tensor_copy(out=mu4[:, i:i + 1], in_=mv[:, 0:1])
                nc.vector.tensor_scalar(out=s4[:, i:i + 1], in0=mv[:, 1:2],
                                        scalar1=EPS, scalar2=None, op0=OP.add)
            y4 = pool.tile([P, LC], F32, tag="ln_y4", bufs=2, name="y4")
            t4 = pool.tile([P, LC], F32, tag="ln_t4", bufs=2, name="t4")
            nc.vector.tensor_scalar(out=t4[:].bitcast(I32), in0=s4[:].bitcast(I32),
                                    scalar1=1, scalar2=None,
                                    op0=OP.arith_shift_right)
            nc.vector.tensor_tensor(out=y4[:].bitcast(I32), in0=magic4[:],
                                    in1=t4[:].bitcast(I32), op=OP.subtract)
            for _ in range(2):
                nc.vector.tensor_tensor(out=t4[:], in0=y4[:], in1=y4[:], op=OP.mult)
                nc.vector.tensor_tensor(out=t4[:], in0=t4[:], in1=s4[:], op=OP.mult)
                nc.vector.tensor_scalar(out=t4[:], in0=t4[:], scalar1=-0.5,
                                        scalar2=1.5, op0=OP.mult, op1=OP.add)
                nc.vector.tensor_tensor(out=y4[:], in0=y4[:], in1=t4[:], op=OP.mult)
            return mu4, y4

        def ln_apply_T(xin, stats, tag, keep_xc=False):
            """normalize (no gamma/beta: gamma folded into weights on host,
            beta re-added downstream) + PE-transpose to channel-major."""
            mu4, y4 = stats
            xnT = pool.tile([P, CC, L], BF16, tag=f"xnT_{tag}",
                            bufs=(2 if tag == "2" else 1), name=f"xnT{tag}")
            xc_full = None
            if keep_xc:
                xc_full = pool.tile([P, LC, C], BF16, tag="xc2", bufs=1,
                                    name="xc2")
            for i in range(LC):
                if keep_xc:
                    xc = xc_full[:, i, :]
                else:
                    xcs = pool.tile([P, C], BF16, tag="xcstage", bufs=2,
                                    name="xcstage")
                    xc = xcs[:]
                nc.vector.tensor_scalar(out=xc, in0=xin[:, i, :],
                                        scalar1=mu4[:, i:i + 1],
                                        scalar2=y4[:, i:i + 1],
                                        op0=OP.subtract, op1=OP.mult)
                transpose_group([xc[:, ts(j, P)] for j in range(CC)],
                                xnT[:, :, ts(i, P)])
            return xnT, xc_full

        def layer_norm_T(xin, tag, keep_xc=False):
            return ln_apply_T(xin, ln_stats(xin), tag, keep_xc)

        def fill_diag(ap_2d, m, val):
            nc.gpsimd.affine_select(out=ap_2d, in_=ap_2d,
                                    compare_op=OP.not_equal, fill=val,
                                    base=P * m, pattern=[[-1, L]],
                                    channel_multiplier=1)

        # ---- hop masks (fp8 0/1, diag filled). masks[b][h] for heads ----
        masks = [[None] * H for _ in range(NB)]

        def mask_base(b):
            """a (fp8, raw) + bf16 copy for transposing, from rel. DVE/ACT
            only, no PE."""
            a8 = pool.tile([P, LC, L], FP8, tag="a8_0", bufs=2, name="a8")
            abf = pool.tile([P, LC, L], BF16, tag="a_bf", bufs=2, name="a_bf")
            for i in range(LC):
                tabs = pool.tile([P, L], BF16, tag="tabs", bufs=1, name="tabs")
                nc.scalar.activation(out=tabs[:], in_=relx[b][:, i, :],
                                     func=ACT.Abs, bias=neg5[:], scale=1.0)
                nc.vector.tensor_scalar(out=a8[:, i, :], in0=tabs[:],
                                        scalar1=4.0, scalar2=None,
                                        op0=OP.is_equal)
                nc.vector.tensor_scalar(out=abf[:, i, :], in0=tabs[:],
                                        scalar1=4.0, scalar2=None,
                                        op0=OP.is_equal)
            return [a8, abf]

        def mask_aT(b, raw):
            """aT8 via PE transposes of the bf16 a copy (fp8 cast on the
            gpsimd copyback)."""
            a8, abf = raw
            aT8 = pool.tile([P, LC, L], FP8, tag="a8_1", bufs=2, name="aT8")
            for i in range(LC):
                transpose_group([abf[:, i, ts(j, P)] for j in range(LC)],
                                aT8[:, :, ts(i, P)])
            raw.append(aT8)

        def mask_mm(b, raw):
            """m2=aTa, m3=aaT via fp8 DoubleRow; then diag-fill all four.
            Emits 16 PE matmuls. Head order: scoresT chunks are [lk, lq], so
            head0 (mask a) uses the transposed tile and head1 the straight
            one; m2/m3 are symmetric."""
            a8, _, aT8 = raw
            # bufs=1: batch 1's mask_mm is emitted after batch 0's S-stage
            # readers of m2/m3
            for idx, src in ((2, a8), (3, aT8)):
                cm = pool.tile([P, LC, L], FP8, tag=f"m{idx}", bufs=1,
                               name=f"m{idx}")
                for m in range(LC):
                    pm = pmm()
                    for k in range(LC // 2):
                        nc.tensor.matmul(pm[:],
                                         src[:, 2 * k:2 * k + 2, ts(m, P)],
                                         src[:, 2 * k:2 * k + 2, :],
                                         start=(k == 0), stop=(k == 1),
                                         perf_mode=DR)
                    nc.vector.tensor_scalar(out=cm[:, m, :], in0=pm[:],
                                            scalar1=0.5, scalar2=None,
                                            op0=OP.is_ge)
                    fill_diag(cm[:, m, :], m, 1.0)
                masks[b][idx] = cm
            for i in range(LC):
                fill_diag(aT8[:, i, :], i, 1.0)
                fill_diag(a8[:, i, :], i, 1.0)
            masks[b][0], masks[b][1] = aT8, a8

        # ---- QKV ----
        qT = [None] * NB
        kT = [None] * NB
        v_sb = [None] * NB

        def qk_block(b, xnT, dst_idx, mc_range):
            """channel-major q/k chunks; dst_idx 0=q, 1=k."""
            dst = qT if dst_idx == 0 else kT
            if dst[b] is None:
                dst[b] = pool.tile([P, CC, L], BF16, tag=f"qk{dst_idx}",
                                   bufs=2, name=f"qk{dst_idx}")
            off = dst_idx * C
            for m in mc_range:
                pm = pmm()
                for k in range(CC):
                    nc.tensor.matmul(pm[:], wq[:, k, off + m * P:off + (m + 1) * P],
                                     xnT[:, k, :],
                                     start=(k == 0), stop=(k == CC - 1))
                nc.vector.tensor_scalar(out=dst[b][:, m, :], in0=pm[:],
                                        scalar1=qkb[:, 4 * dst_idx + m:
                                                    4 * dst_idx + m + 1],
                                        scalar2=None, op0=OP.add)

        def v_block(b, xnT, mt_range):
            # bufs=1: batch 1's v is computed after batch 0's A-stages
            if v_sb[b] is None:
                v_sb[b] = pool.tile([P, LC, C], BF16, tag="v_sb", bufs=1,
                                    name="v_sb")
            for m in mt_range:
                pm = pmm()
                for k in range(CC):
                    nc.tensor.matmul(pm[:], xnT[:, k, ts(m, P)],
                                     wq[:, k, 2 * C:3 * C],
                                     start=(k == 0), stop=(k == CC - 1))
                nc.vector.tensor_tensor(out=v_sb[b][:, m, :], in0=pm[:],
                                        in1=vb[:], op=OP.add)

        # ---- attention head stages ----
        def S(b, h, atts):
            """scores + exp + mask-mult for head h -> attnT tile."""
            attnT = pool.tile([P, LC, L], BF16, tag="attnT", bufs=2,
                              name="attnT")
            atts[h] = attnT
            for i in range(LC):
                pm = pmm()
                nc.tensor.matmul(pm[:], kT[b][:, h, ts(i, P)], qT[b][:, h, :],
                                 start=True, stop=True)
                nc.scalar.activation(out=attnT[:, i, :], in_=pm[:],
                                     func=ACT.Exp, scale=INV_SQRT_HS)
                nc.vector.tensor_tensor(out=attnT[:, i, :], in0=attnT[:, i, :],
                                        in1=masks[b][h][:, i, :], op=OP.mult)

        def D(b, h, atts, rbcs):
            """denominator: fold chunks on DVE, one ones-matmul, recip, bcast."""
            at = atts[h]
            fold = pool.tile([P, L], BF16, tag="fold", bufs=2, name="fold")
            nc.vector.tensor_tensor(out=fold[:], in0=at[:, 0, :], in1=at[:, 1, :],
                                    op=OP.add)
            nc.vector.tensor_tensor(out=fold[:], in0=fold[:], in1=at[:, 2, :],
                                    op=OP.add)
            nc.vector.tensor_tensor(out=fold[:], in0=fold[:], in1=at[:, 3, :],
                                    op=OP.add)
            pd = psum.tile([1, L], F32, tag="dn", bufs=1, name="pd")
            nc.tensor.matmul(pd[:], ones_b[:], fold[:], start=True, stop=True)
            recip = pool.tile([1, L], F32, tag="recip", bufs=2, name="recip")
            nc.vector.reciprocal_approx_fast(out=recip[:], in_=pd[:])
            rbc = pool.tile([P, L], F32, tag="rbc", bufs=2, name="rbc")
            nc.gpsimd.partition_broadcast(rbc[:], recip[:])
            rbcs[h] = rbc

        def A(b, h, atts, rbcs, OT):
            po = pmm()
            for i in range(LC):
                nc.tensor.matmul(po[:], v_sb[b][:, i, ts(h, P)],
                                 atts[h][:, i, :],
                                 start=(i == 0), stop=(i == LC - 1))
            nc.vector.tensor_tensor(out=OT[:, h, :], in0=po[:],
                                    in1=rbcs[h][:], op=OP.mult)

        x1 = [None] * NB

        def proj(b, x_tile, OT):
            x1[b] = pool.tile([P, LC, C], BF16, tag="x1", bufs=2, name="x1")
            for m in range(LC):
                pm = pmm()
                for k in range(CC):
                    nc.tensor.matmul(pm[:], OT[:, k, ts(m, P)], wp[:, k, :],
                                     start=(k == 0), stop=(k == CC - 1))
                nc.vector.tensor_tensor(out=x1[b][:, m, :], in0=x_tile[:, m, :],
                                        in1=pm[:], op=OP.add)

        # ---- GIN ----
        u1T = [None] * NB
        u2T = [None] * NB

        def u_block(b, xc2, uidx, mc_range):
            """uT = ((adj|adjT) @ xn2)^T with the LN2-beta rank-1 term fused
            into the copyback: u += adj_rowsum[l] * beta2[c]."""
            lst = u1T if uidx == 0 else u2T
            # bufs=1: batch 1's u is emitted after batch 0's hT readers
            if lst[b] is None:
                lst[b] = pool.tile([P, CC, L], BF16, tag=f"u{uidx}", bufs=1,
                                   name=f"u{uidx}")
            rhs = adjT_sb[b] if uidx == 0 else adj_sb[b]
            for m in mc_range:
                pm = pmm()
                for k in range(LC):
                    nc.tensor.matmul(pm[:], xc2[:, k, ts(m, P)], rhs[:, k, :],
                                     start=(k == 0), stop=(k == LC - 1))
                nc.vector.scalar_tensor_tensor(out=lst[b][:, m, :],
                                               in0=adjsum_bc[b][:, ts(uidx, L)],
                                               scalar=ln2b[:, m:m + 1],
                                               in1=pm[:],
                                               op0=OP.mult, op1=OP.add)

        hT = [None] * NB

        def hT_block(b, xn2T, mh_range):
            if hT[b] is None:
                hT[b] = pool.tile([P, HC, L], BF16, tag="hT", bufs=1, name="hT")
            for mh in mh_range:
                pm = pmm()
                uT = u1T[b] if mh < HC // 2 else u2T[b]
                for k in range(CC):
                    nc.tensor.matmul(pm[:], wgc[:, k, ts(mh, P)], uT[:, k, :],
                                     start=(k == 0), stop=False)
                for k in range(CC):
                    nc.tensor.matmul(pm[:], wf1[:, k, ts(mh, P)], xn2T[:, k, :],
                                     start=False, stop=(k == CC - 1))
                nc.scalar.activation(out=hT[b][:, mh, :], in_=pm[:],
                                     func=ACT.Relu, bias=fc1b[:, mh:mh + 1],
                                     scale=1.0)

        def fc2_block(b, mt_range, wf2):
            for m in mt_range:
                pm = pmm()
                for k in range(HC):
                    nc.tensor.matmul(pm[:], hT[b][:, k, ts(m, P)], wf2[:, k, :],
                                     start=(k == 0), stop=(k == HC - 1))
                o_sb = pool.tile([P, C], F32, tag="o_sb", bufs=2, name="o_sb")
                nc.vector.tensor_tensor(out=o_sb[:], in0=x1[b][:, m, :],
                                        in1=pm[:], op=OP.add)
                nc.sync.dma_start(out=out_t3[b][:, m, :], in_=o_sb[:])

        # ================= schedule =================
        # batch 0 front: LN1 + masks + QKV
        xn1T_0, _ = layer_norm_T(x_t[0], "1")
        raw0 = mask_base(0)
        # late-emitted input DMAs: queue position is after wq/wp/x1, and the
        # relx tag-rotation WAR (bufs=2) sees batch 0's readers above
        dma_rel(1)
        dma_adj(0)
        wgc, wf1 = dma_gin_weights()
        mask_aT(0, raw0)
        mask_mm(0, raw0)
        for m in range(CC):
            qk_block(0, xn1T_0, 0, [m])
            qk_block(0, xn1T_0, 1, [m])
        v_block(0, xn1T_0, range(LC))

        # batch 1 LN + mask DVE prep before batch 0 heads
        xn1T_1, _ = layer_norm_T(x_t[1], "1")
        raw1 = mask_base(1)

        # batch 0 heads; fillers: adjT b0, b1 QKV, b1 mask transposes
        atts, rbcs = {}, {}
        OT0 = pool.tile([P, H, L], BF16, tag="OT", bufs=1, name="OT")
        S(0, 0, atts)
        S(0, 1, atts)
        D(0, 0, atts, rbcs)
        adjT_transpose(0)                     # F1: 16 transposes
        A(0, 0, atts, rbcs, OT0)
        D(0, 1, atts, rbcs)
        S(0, 2, atts)
        qk_block(1, xn1T_1, 0, range(CC))     # F2: 16 mm
        A(0, 1, atts, rbcs, OT0)
        D(0, 2, atts, rbcs)
        S(0, 3, atts)
        qk_block(1, xn1T_1, 1, range(CC))     # F3: 16 mm
        A(0, 2, atts, rbcs, OT0)
        D(0, 3, atts, rbcs)
        mask_aT(1, raw1)                      # F4: 16 transposes
        A(0, 3, atts, rbcs, OT0)
        v_block(1, xn1T_1, range(2))          # F5: 8 mm (covers OT0 drain)
        proj(0, x_t[0], OT0)

        # post-proj0 stretch: b1 mask matmuls (after S(0,2)/S(0,3) since
        # m2/m3 are bufs=1), rest of v, LN2 b0 stats (DVE only); the LN2
        # transposes go after the first b1 scores
        mask_mm(1, raw1)
        v_block(1, xn1T_1, range(2, LC))
        ln2_0_stats = ln_stats(x1[0])
        atts, rbcs = {}, {}
        OT1 = pool.tile([P, H, L], BF16, tag="OT", bufs=1, name="OT")
        S(1, 0, atts)
        S(1, 1, atts)
        xn2T_0, xc2_0 = ln_apply_T(x1[0], ln2_0_stats, "2", keep_xc=True)
        D(1, 0, atts, rbcs)
        u_block(0, xc2_0, 0, range(CC))       # F1: 16 mm
        A(1, 0, atts, rbcs, OT1)
        D(1, 1, atts, rbcs)
        S(1, 2, atts)
        u_block(0, xc2_0, 1, range(CC))       # F2: 16 mm
        A(1, 1, atts, rbcs, OT1)
        D(1, 2, atts, rbcs)
        S(1, 3, atts)
        # adj b1 DMA: emitted after all adj b0 readers (u_block above)
        dma_adj(1)
        hT_block(0, xn2T_0, range(0, 2))      # F3: 16 mm
        A(1, 2, atts, rbcs, OT1)
        D(1, 3, atts, rbcs)
        hT_block(0, xn2T_0, range(2, 4))      # F4: 16 mm
        A(1, 3, atts, rbcs, OT1)
        hT_block(0, xn2T_0, range(4, 8))      # F5: 32 mm (covers OT1 drain)
        proj(1, x_t[1], OT1)
        # attention weights are dead now; close their pool and stream wf2
        # into the freed region (needed ~25us later by fc2_block(0))
        wA_cm.__exit__(None, None, None)
        with tc.tile_pool(name="wB", bufs=1) as wB:
            wf2 = wB.tile([P, HC, C], BF16, name="wf2")
            nc.sync.dma_start(out=wf2[:],
                              in_=wfc2_d.rearrange("(ko p) n -> p ko n", p=P))

            # batch 1 LN2: stats under hT b0, transposes after
            ln2_1_stats = ln_stats(x1[1])
            hT_block(0, xn2T_0, range(8, 12))
            xn2T_1, xc2_1 = ln_apply_T(x1[1], ln2_1_stats, "2", keep_xc=True)
            hT_block(0, xn2T_0, range(12, HC))
            adjT_transpose(1)
            u_block(1, xc2_1, 0, range(CC))
            u_block(1, xc2_1, 1, range(CC))
            fc2_block(0, range(LC), wf2)
            hT_block(1, xn2T_1, range(HC))
            fc2_block(1, range(LC), wf2)


# ======================= SPMD wrapper =======================
N_CORES = 8
_CACHE = {}


def _get_program():
    if "nc" not in _CACHE:
        from concourse import bacc
        nc = bacc.Bacc("TRN2", target_bir_lowering=False, debug=False,
                       num_devices=N_CORES)
        build_encoder_program(nc)
        nc.finalize()
        _CACHE["nc"] = nc
    return _CACHE["nc"]


def prep_in_maps(inputs):
    """Host-side prep: cast to bf16, fold LN gammas into the consuming
    weights, precompute LN-beta bias rows and adj row/col sums."""
    BF = ml_dtypes.bfloat16
    f32 = np.float32
    g = lambda k: np.asarray(inputs[k], f32)
    x, rel, adj = g("x"), g("rel_pos"), g("adj")
    g1, b1 = g("ln1_g"), g("ln1_b")
    g2, b2 = g("ln2_g"), g("ln2_b")
    wqkv, wproj = g("w_qkv"), g("w_proj")
    wfc1, wgcn, wfc2 = g("w_fc1"), g("w_gcn"), g("w_fc2")

    qkvb = b1 @ wqkv                      # [3C]
    shared = {
        "wqkv": np.ascontiguousarray((g1[:, None] * wqkv).astype(BF)),
        "wproj": np.ascontiguousarray(wproj.astype(BF)),
        "wgcn": np.ascontiguousarray((g2[:, None] * wgcn).astype(BF)),
        "wfc1": np.ascontiguousarray((g2[:, None] * wfc1).astype(BF)),
        "wfc2": np.ascontiguousarray(wfc2.astype(BF)),
        "qkb": np.ascontiguousarray(qkvb[:2 * C].reshape(2 * CC, P).T.astype(f32)),
        "vbr": np.ascontiguousarray(qkvb[None, 2 * C:].astype(BF)),
        "fc1b": np.ascontiguousarray((b2 @ wfc1).reshape(HC, P).T.astype(f32)),
        "ln2b": np.ascontiguousarray(b2.reshape(CC, P).T.astype(f32)),
    }
    in_maps = []
    for c in range(N_CORES):
        sl = slice(NB * c, NB * (c + 1))
        xs, rs, ads = x[sl], rel[sl], adj[sl]
        m = dict(shared)
        m["x"] = np.ascontiguousarray(xs.astype(BF))
        m["rel"] = np.ascontiguousarray(rs.astype(BF))
        m["adj"] = np.ascontiguousarray(ads.astype(BF))
        m["adjsum"] = np.ascontiguousarray(
            np.stack([ads.sum(2), ads.sum(1)], axis=1)
            .reshape(NB, 1, 2 * L).astype(BF))
        in_maps.append(m)
    return in_maps


def kernel(**inputs):
    """Full-input entry point: shards batch dim over 8 NeuronCores,
    runs the Bass program, gathers the full output."""
    from concourse.bass_utils import run_bass_kernel_spmd

    nc = _get_program()
    B = inputs["x"].shape[0]
    assert B == NB * N_CORES, f"expected B={NB * N_CORES}, got {B}"
    in_maps = prep_in_maps(inputs)
    res = run_bass_kernel_spmd(nc, in_maps, list(range(N_CORES)))
    return np.concatenate([res.results[c]["out"] for c in range(N_CORES)], axis=0)
